# revision 5
# baseline (speedup 1.0000x reference)
"""HGT (heterogeneous graph transformer) 2-layer + SDDMM kernel for 8 trn2 cores.

Strategy:
  - Nodes row-sharded: core c owns rows [c*2048, (c+1)*2048) of both node types.
  - Dense projections (initial lin+relu, per-layer Q/K/V, update) computed on the
    owned rows in feature-major layout; Q/KV gather tables assembled full-size on
    every core with one AllGather per layer.
  - Edges sharded by destination: core c processes every edge whose dst is in its
    row range, sub-bucketed by 128-node destination tile.  Per 128-edge sub-chunk
    a one-hot [edge, node] matrix (iota==dst compare) is matmul'd against
    [exp(score) | v*exp(score)] and accumulated in PSUM -> exact segment softmax
    without a segment max (scores are O(1), exp cannot overflow).
  - Final SDDMM: pairs sharded across cores, gather Em/Ed rows, multiply+reduce.

Host-side work is limited to sharding/packing of indices and folding the small
parameter matrices (relation transforms, softmax scale, skip gate) into the
projection weights.
"""

import sys

for _p in ("/opt/trn_rl_repo",):
    if _p not in sys.path:
        sys.path.insert(0, _p)

import numpy as np

import concourse.bacc as bacc
import concourse.bass as bass
import concourse.mybir as mybir
import concourse.tile as tile
from concourse.masks import make_identity
from concourse.bass_utils import run_bass_kernel_spmd

F32 = mybir.dt.float32
I16 = mybir.dt.int16
I32 = mybir.dt.int32
AF = mybir.ActivationFunctionType
ALU = mybir.AluOpType

# problem constants (hardcoded per the task contract)
N = 16384
F_IN = 256
HID = 128
H = 4
DH = 32
LAYERS = 2
E = 400000
P = 200000
CORES = 8

EDGE_CH = 1024  # edges gathered per dma_gather call
TS = 128  # destination-node tile size


def _roundup(x, m):
    return (x + m - 1) // m * m


def _wrap16(arr):
    """idx i -> [i % 16, i // 16], replicated to 128 partitions."""
    w = arr.reshape(-1, 16).T  # [16, L/16]
    return np.ascontiguousarray(np.tile(w, (8, 1)))  # [128, L/16]


def _wrap128(arr):
    """value i -> [i % 128, i // 128]."""
    return np.ascontiguousarray(arr.reshape(-1, 128).T)


def _blockdiag(rel):
    """rel [H, DH, DH] -> [HID, HID] block diagonal."""
    out = np.zeros((HID, HID), np.float32)
    for h in range(H):
        out[h * DH : (h + 1) * DH, h * DH : (h + 1) * DH] = rel[h]
    return out


def prep_weights(params):
    """Fold params into the matrices the kernel consumes. Returns dict name->np."""
    w = {}
    sd = np.float32(1.0 / np.sqrt(DH))

    for i, t in enumerate(("n1", "n2")):
        lin = params[f"lin_{t}"]
        w[f"w0_{i}"] = np.asarray(lin["W"], np.float32)  # [256, 128]
        w[f"b0_{i}"] = np.asarray(lin["b"], np.float32).reshape(HID, 1)

    betas = []  # [layer][type]
    for l, lp in enumerate(params["layers"]):
        lbetas = []
        for i, (t, et) in enumerate((("n1", "e21"), ("n2", "e12"))):
            # type i is the *destination* of direction et; its K/V are used as the
            # *source* of the opposite direction.
            tp = lp[t]
            src_et = "e12" if i == 0 else "e21"  # direction where type i is src
            rel_src = lp[src_et]
            a = np.asarray(rel_src["a_rel"], np.float32) * (
                np.asarray(rel_src["p_rel"], np.float32)[:, None, None] * sd
            )
            bda = _blockdiag(a)
            bdm = _blockdiag(np.asarray(rel_src["m_rel"], np.float32))
            wk = np.asarray(tp["k"]["W"], np.float32)
            bk = np.asarray(tp["k"]["b"], np.float32)
            wv = np.asarray(tp["v"]["W"], np.float32)
            bv = np.asarray(tp["v"]["b"], np.float32)
            w[f"wkv{i}_{l}"] = np.ascontiguousarray(
                np.concatenate([wk @ bda, wv @ bdm], axis=1)
            )  # [128, 256]
            bkv = np.concatenate([bk @ bda, bv @ bdm])  # [256]
            w[f"bkv{i}_{l}"] = np.ascontiguousarray(bkv.reshape(2, HID).T)  # [128, 2]
            w[f"wq{i}_{l}"] = np.asarray(tp["q"]["W"], np.float32)
            w[f"bq{i}_{l}"] = np.asarray(tp["q"]["b"], np.float32).reshape(HID, 1)
            alpha = 1.0 / (1.0 + np.exp(-float(np.asarray(tp["skip"]))))
            w[f"wa{i}_{l}"] = np.asarray(tp["a"]["W"], np.float32) * np.float32(alpha)
            w[f"ba{i}_{l}"] = (
                np.asarray(tp["a"]["b"], np.float32) * np.float32(alpha)
            ).reshape(HID, 1)
            lbetas.append(float(1.0 - alpha))
        betas.append(lbetas)
    return w, betas


def prep_edges(edge, n_cores, nloc):
    """Bucket edges by (dst core, dst 128-tile). Returns per-core packed arrays."""
    src = np.asarray(edge[0], np.int64)
    dst = np.asarray(edge[1], np.int64)
    ntile = nloc // TS
    bucket = dst // TS  # global dst tile id == core * ntile + tile
    order = np.argsort(bucket, kind="stable")
    src_s, dst_s = src[order], dst[order]
    counts = np.bincount(bucket, minlength=n_cores * ntile)
    pad = max(TS, _roundup(int(counts.max()), TS))
    starts = np.zeros(n_cores * ntile + 1, np.int64)
    np.cumsum(counts, out=starts[1:])

    per_core = []
    for c in range(n_cores):
        src_pad = np.zeros((ntile, pad), np.int16)
        dst_pad = np.zeros((ntile, pad), np.int16)
        dloc_pad = np.full((ntile, pad), -1.0, np.float32)
        for t in range(ntile):
            b = c * ntile + t
            s0, s1 = starts[b], starts[b + 1]
            n = s1 - s0
            src_pad[t, :n] = src_s[s0:s1]
            dst_pad[t, :n] = dst_s[s0:s1]
            dloc_pad[t, :n] = (dst_s[s0:s1] - (c * nloc + t * TS)).astype(np.float32)
        per_core.append(
            {
                "src16": _wrap16(src_pad.reshape(-1)),
                "dst16": _wrap16(dst_pad.reshape(-1)),
                "dstloc": _wrap128(dloc_pad.reshape(-1)),
            }
        )
    return per_core, pad


def prep_pairs(edge_index, n_cores):
    m = np.asarray(edge_index[0], np.int64)
    d = np.asarray(edge_index[1], np.int64)
    npairs = m.shape[0]
    pc = _roundup(npairs, n_cores) // n_cores
    padp = _roundup(pc, TS)
    per_core = []
    for c in range(n_cores):
        mm = np.zeros(padp, np.int16)
        dd = np.zeros(padp, np.int16)
        sl = slice(c * pc, min((c + 1) * pc, npairs))
        n = sl.stop - sl.start
        mm[:n] = m[sl]
        dd[:n] = d[sl]
        per_core.append({"m16": _wrap16(mm), "d16": _wrap16(dd)})
    return per_core, padp, pc


def build_program(cfg):
    """Build the SPMD bass program. cfg: dict with sizes + betas."""
    C = cfg["C"]
    NLOC = cfg["NLOC"]
    NTOT = C * NLOC
    FIN = cfg["FIN"]
    L = cfg["L"]
    PAD = cfg["PAD"]  # per direction [2]
    PADP = cfg["PADP"]
    betas = cfg["betas"]
    NT = NLOC // TS
    KF = FIN // 128

    nc = bacc.Bacc(
        "TRN2", target_bir_lowering=False, debug=False, num_devices=C
    )

    ext = {}

    def inp(name, shape, dt=F32):
        ext[name] = nc.dram_tensor(name, shape, dt, kind="ExternalInput")
        return ext[name]

    xts = [inp(f"x{t}t", [FIN, NLOC]) for t in range(2)]
    for t in range(2):
        inp(f"w0_{t}", [FIN, HID])
        inp(f"b0_{t}", [HID, 1])
        for l in range(L):
            inp(f"wq{t}_{l}", [HID, HID])
            inp(f"bq{t}_{l}", [HID, 1])
            inp(f"wkv{t}_{l}", [HID, 2 * HID])
            inp(f"bkv{t}_{l}", [HID, 2])
            inp(f"wa{t}_{l}", [HID, HID])
            inp(f"ba{t}_{l}", [HID, 1])
    for d in range(2):
        inp(f"src16_{d}", [128, NT * PAD[d] // 16], I16)
        inp(f"dst16_{d}", [128, NT * PAD[d] // 16], I16)
        inp(f"dstloc_{d}", [128, NT * PAD[d] // 128])
    inp("m16", [128, PADP // 16], I16)
    inp("d16", [128, PADP // 16], I16)

    y_out = nc.dram_tensor("y", [128, PADP // 128], F32, kind="ExternalOutput")

    # internal DRAM ("Shared" collective outputs only supported for >4 cores)
    shared = {"addr_space": "Shared"} if C > 4 else {}
    agin = [nc.dram_tensor(f"agin_{l}", [NLOC, 768], F32) for l in range(L)]
    agout = [
        nc.dram_tensor(f"agout_{l}", [NTOT, 768], F32, **shared) for l in range(L)
    ]
    fin_in = nc.dram_tensor("fin_in", [NLOC, 2 * L * HID], F32)
    fin_out = nc.dram_tensor("fin_out", [NTOT, 2 * L * HID], F32, **shared)

    rg = [list(range(C))]

    with tile.TileContext(nc) as tc:
        with (
            tc.tile_pool(name="const", bufs=1) as const,
            tc.tile_pool(name="wp", bufs=1) as wp,
            tc.tile_pool(name="hp", bufs=2) as hp,
            tc.tile_pool(name="work", bufs=3) as work,
            tc.tile_pool(name="ep", bufs=2) as ep,
            tc.tile_pool(name="idxp", bufs=1) as idxp,
            tc.tile_pool(name="pp", bufs=2, space="PSUM") as pp,
            tc.tile_pool(name="pacc", bufs=2, space="PSUM") as pacc,
        ):
            # ---- constants ----
            iota_i = const.tile([128, 128], I32, tag="iota_i")
            nc.gpsimd.iota(iota_i[:], pattern=[[1, 128]], base=0, channel_multiplier=0)
            iota_f = const.tile([128, 128], F32, tag="iota_f")
            nc.vector.tensor_copy(out=iota_f[:], in_=iota_i[:])
            ident = const.tile([128, 128], F32, tag="ident")
            make_identity(nc, ident[:])

            def load_tile(name, shape, dt=F32, src=None):
                t = wp.tile(shape, dt, tag=name)
                nc.sync.dma_start(out=t[:], in_=src if src is not None else ext[name][:])
                return t

            w0 = [
                [
                    load_tile(f"w0_{t}_{k}", [128, HID], src=ext[f"w0_{t}"][k * 128 : (k + 1) * 128, :])
                    for k in range(KF)
                ]
                for t in range(2)
            ]
            b0 = [load_tile(f"b0_{t}", [HID, 1]) for t in range(2)]
            wq = [[load_tile(f"wq{t}_{l}", [HID, HID]) for l in range(L)] for t in range(2)]
            bq = [[load_tile(f"bq{t}_{l}", [HID, 1]) for l in range(L)] for t in range(2)]
            wkv = [[load_tile(f"wkv{t}_{l}", [HID, 2 * HID]) for l in range(L)] for t in range(2)]
            bkv = [[load_tile(f"bkv{t}_{l}", [HID, 2]) for l in range(L)] for t in range(2)]
            wa = [[load_tile(f"wa{t}_{l}", [HID, HID]) for l in range(L)] for t in range(2)]
            ba = [[load_tile(f"ba{t}_{l}", [HID, 1]) for l in range(L)] for t in range(2)]

            src16 = [
                load_tile(f"src16_{d}", [128, NT * PAD[d] // 16], I16) for d in range(2)
            ]
            dst16 = [
                load_tile(f"dst16_{d}", [128, NT * PAD[d] // 16], I16) for d in range(2)
            ]
            dstloc = [
                load_tile(f"dstloc_{d}", [128, NT * PAD[d] // 128]) for d in range(2)
            ]
            m16 = load_tile("m16", [128, PADP // 16], I16)
            d16 = load_tile("d16", [128, PADP // 16], I16)

            # ---- initial projection (feature-major h tiles) ----
            h = []
            for t in range(2):
                ht = hp.tile([128, NLOC], F32, tag=f"h{t}")
                for j in range(NT):
                    cols = slice(j * 128, (j + 1) * 128)
                    ps = pp.tile([128, 128], F32, tag="dps")
                    for k in range(KF):
                        xls = work.tile([128, 128], F32, tag="xls")
                        nc.sync.dma_start(
                            out=xls[:],
                            in_=xts[t][k * 128 : (k + 1) * 128, cols],
                        )
                        nc.tensor.matmul(
                            out=ps[:],
                            lhsT=w0[t][k][:],
                            rhs=xls[:],
                            start=(k == 0),
                            stop=(k == KF - 1),
                        )
                    nc.scalar.activation(
                        out=ht[:, cols], in_=ps[:], func=AF.Relu, bias=b0[t][:, 0:1]
                    )
                h.append(ht)

            # ---- layers ----
            for l in range(L):
                # projections -> node-major tables in agin
                for t in range(2):
                    base = 0 if t == 0 else 384
                    for j in range(NT):
                        cols = slice(j * 128, (j + 1) * 128)
                        rows = slice(j * 128, (j + 1) * 128)
                        # q
                        ps = pp.tile([128, 128], F32, tag="dps")
                        nc.tensor.matmul(
                            out=ps[:], lhsT=wq[t][l][:], rhs=h[t][:, cols],
                            start=True, stop=True,
                        )
                        qs = work.tile([128, 128], F32, tag="qs")
                        nc.scalar.activation(
                            out=qs[:], in_=ps[:], func=AF.Identity, bias=bq[t][l][:, 0:1]
                        )
                        pt = pp.tile([128, 128], F32, tag="tps")
                        nc.tensor.transpose(out=pt[:], in_=qs[:], identity=ident[:])
                        qn = work.tile([128, 128], F32, tag="qn")
                        nc.vector.tensor_copy(out=qn[:], in_=pt[:])
                        nc.sync.dma_start(
                            out=agin[l][rows, base : base + 128], in_=qn[:]
                        )
                        # kv halves
                        for hh in range(2):
                            ps2 = pp.tile([128, 128], F32, tag="dps")
                            nc.tensor.matmul(
                                out=ps2[:],
                                lhsT=wkv[t][l][:, hh * 128 : (hh + 1) * 128],
                                rhs=h[t][:, cols],
                                start=True,
                                stop=True,
                            )
                            ks = work.tile([128, 128], F32, tag="qs")
                            nc.scalar.activation(
                                out=ks[:], in_=ps2[:], func=AF.Identity,
                                bias=bkv[t][l][:, hh : hh + 1],
                            )
                            pt2 = pp.tile([128, 128], F32, tag="tps")
                            nc.tensor.transpose(out=pt2[:], in_=ks[:], identity=ident[:])
                            kn = work.tile([128, 128], F32, tag="qn")
                            nc.vector.tensor_copy(out=kn[:], in_=pt2[:])
                            nc.sync.dma_start(
                                out=agin[l][rows, base + 128 + hh * 128 : base + 256 + hh * 128],
                                in_=kn[:],
                            )

                nc.gpsimd.collective_compute(
                    "AllGather",
                    ALU.bypass,
                    replica_groups=rg,
                    ins=[agin[l][:, :]],
                    outs=[agout[l][:, :]],
                )

                # edge phase per direction
                hnew = [None, None]
                for d in range(2):
                    # d=0: edge_12 (src type0 KV, dst type1 Q) -> agg for type1
                    kv_col = 128 if d == 0 else 512
                    q_col = 384 if d == 0 else 0
                    ut_t = 1 if d == 0 else 0
                    kv_tab = agout[l][:, kv_col : kv_col + 256]
                    q_tab = agout[l][:, q_col : q_col + 128]
                    padd = PAD[d]
                    hn = hp.tile([128, NLOC], F32, tag=f"h{ut_t}")

                    for j in range(NT):
                        acc = pacc.tile([128, 132], F32, tag="acc")
                        mm_i = 0
                        total_mm = padd // 128
                        for ch0 in range(0, padd, EDGE_CH):
                            ni = min(EDGE_CH, padd - ch0)
                            nb = ni // 128
                            icol = (j * padd + ch0) // 16
                            kvt = ep.tile([128, EDGE_CH // 128, 256], F32, tag="kv")
                            qt = ep.tile([128, EDGE_CH // 128, 128], F32, tag="qg")
                            nc.gpsimd.dma_gather(
                                out_ap=kvt[:, :nb, :],
                                in_ap=kv_tab,
                                idxs_ap=src16[d][:, icol : icol + ni // 16],
                                num_idxs=ni,
                                num_idxs_reg=ni,
                                elem_size=256,
                                elem_step=768,
                            )
                            nc.gpsimd.dma_gather(
                                out_ap=qt[:, :nb, :],
                                in_ap=q_tab,
                                idxs_ap=dst16[d][:, icol : icol + ni // 16],
                                num_idxs=ni,
                                num_idxs_reg=ni,
                                elem_size=128,
                                elem_step=768,
                            )
                            tmp = ep.tile([128, EDGE_CH // 128, 128], F32, tag="tmp")
                            nc.vector.tensor_mul(
                                out=tmp[:, :nb, :],
                                in0=kvt[:, :nb, 0:128],
                                in1=qt[:, :nb, :],
                            )
                            sc = ep.tile([128, EDGE_CH // 128, H], F32, tag="sc")
                            nc.vector.tensor_reduce(
                                out=sc[:, :nb, :],
                                in_=tmp[:, :nb, :].rearrange(
                                    "p a (h d) -> p a h d", d=DH
                                ),
                                axis=mybir.AxisListType.X,
                                op=ALU.add,
                            )
                            ut = ep.tile([128, EDGE_CH // 128, 4 + HID], F32, tag="ut")
                            nc.scalar.activation(
                                out=ut[:, :nb, 0:4], in_=sc[:, :nb, :], func=AF.Exp
                            )
                            nc.vector.tensor_mul(
                                out=ut[:, :nb, 4 : 4 + HID].rearrange(
                                    "p a (h d) -> p a h d", d=DH
                                ),
                                in0=kvt[:, :nb, 128:256].rearrange(
                                    "p a (h d) -> p a h d", d=DH
                                ),
                                in1=ut[:, :nb, 0:4].to_broadcast(
                                    [128, nb, H, DH]
                                ),
                            )
                            for b in range(nb):
                                onb = ep.tile([128, 128], F32, tag="onh")
                                ccol = j * (padd // 128) + ch0 // 128 + b
                                nc.vector.tensor_scalar(
                                    out=onb[:],
                                    in0=iota_f[:],
                                    scalar1=dstloc[d][:, ccol : ccol + 1],
                                    scalar2=None,
                                    op0=ALU.is_equal,
                                )
                                nc.tensor.matmul(
                                    out=acc[:],
                                    lhsT=onb[:],
                                    rhs=ut[:, b, :],
                                    start=(mm_i == 0),
                                    stop=(mm_i == total_mm - 1),
                                )
                                mm_i += 1
                        # drain destination tile j
                        sres = ep.tile([128, 4], F32, tag="sres")
                        nc.vector.tensor_scalar_add(
                            out=sres[:], in0=acc[:, 0:4], scalar1=1e-16
                        )
                        rec = ep.tile([128, 4], F32, tag="rec")
                        nc.vector.reciprocal(out=rec[:], in_=sres[:])
                        aggn = ep.tile([128, 128], F32, tag="aggn")
                        nc.vector.tensor_mul(
                            out=aggn[:].rearrange("p (h d) -> p h d", d=DH),
                            in0=acc[:, 4 : 4 + HID].rearrange("p (h d) -> p h d", d=DH),
                            in1=rec[:].to_broadcast([128, H, DH]),
                        )
                        # update: h_new = gelu(agg) @ wa' + ba' + beta*h
                        ptA = pp.tile([128, 128], F32, tag="tps")
                        nc.tensor.transpose(out=ptA[:], in_=aggn[:], identity=ident[:])
                        gel = work.tile([128, 128], F32, tag="gel")
                        nc.scalar.activation(out=gel[:], in_=ptA[:], func=AF.Gelu)
                        pu = pp.tile([128, 128], F32, tag="dps")
                        nc.tensor.matmul(
                            out=pu[:], lhsT=wa[ut_t][l][:], rhs=gel[:],
                            start=True, stop=True,
                        )
                        tmpu = work.tile([128, 128], F32, tag="tmpu")
                        nc.scalar.activation(
                            out=tmpu[:], in_=pu[:], func=AF.Identity,
                            bias=ba[ut_t][l][:, 0:1],
                        )
                        cols = slice(j * 128, (j + 1) * 128)
                        nc.vector.scalar_tensor_tensor(
                            out=hn[:, cols],
                            in0=h[ut_t][:, cols],
                            scalar=betas[l][ut_t],
                            in1=tmpu[:],
                            op0=ALU.mult,
                            op1=ALU.add,
                        )
                        # node-major copy for the final SDDMM tables
                        ptB = pp.tile([128, 128], F32, tag="tps")
                        nc.tensor.transpose(out=ptB[:], in_=hn[:, cols], identity=ident[:])
                        nmh = work.tile([128, 128], F32, tag="qn")
                        nc.vector.tensor_copy(out=nmh[:], in_=ptB[:])
                        fcol = (l * 128) if ut_t == 0 else (L * 128 + l * 128)
                        nc.sync.dma_start(
                            out=fin_in[cols, fcol : fcol + 128], in_=nmh[:]
                        )
                    hnew[ut_t] = hn
                h = hnew

            # ---- final AllGather + SDDMM ----
            nc.gpsimd.collective_compute(
                "AllGather",
                ALU.bypass,
                replica_groups=rg,
                ins=[fin_in[:, :]],
                outs=[fin_out[:, :]],
            )
            em_tab = fin_out[:, 0 : L * 128]
            ed_tab = fin_out[:, L * 128 : 2 * L * 128]
            EMW = L * 128
            ytile = work.tile([128, PADP // 128], F32, tag="yt")
            for ch0 in range(0, PADP, EDGE_CH):
                ni = min(EDGE_CH, PADP - ch0)
                nb = ni // 128
                emg = ep.tile([128, EDGE_CH // 128, EMW], F32, tag="kv")
                edg = ep.tile([128, EDGE_CH // 128, EMW], F32, tag="qg")
                nc.gpsimd.dma_gather(
                    out_ap=emg[:, :nb, :],
                    in_ap=em_tab,
                    idxs_ap=m16[:, ch0 // 16 : (ch0 + ni) // 16],
                    num_idxs=ni,
                    num_idxs_reg=ni,
                    elem_size=EMW,
                    elem_step=2 * EMW,
                )
                nc.gpsimd.dma_gather(
                    out_ap=edg[:, :nb, :],
                    in_ap=ed_tab,
                    idxs_ap=d16[:, ch0 // 16 : (ch0 + ni) // 16],
                    num_idxs=ni,
                    num_idxs_reg=ni,
                    elem_size=EMW,
                    elem_step=2 * EMW,
                )
                prod = ep.tile([128, EDGE_CH // 128, EMW], F32, tag="tmp")
                nc.vector.tensor_mul(
                    out=prod[:, :nb, :], in0=emg[:, :nb, :], in1=edg[:, :nb, :]
                )
                nc.vector.tensor_reduce(
                    out=ytile[:, ch0 // 128 : ch0 // 128 + nb],
                    in_=prod[:, :nb, :],
                    axis=mybir.AxisListType.X,
                    op=ALU.add,
                )
            nc.sync.dma_start(out=y_out[:], in_=ytile[:])

    nc.compile()
    return nc


def prepare(x_n1, x_n2, edge_12, edge_21, edge_index, params, n_cores=CORES):
    """Host-side prep: returns (cfg, in_maps, meta) for build/run/unshard."""
    x1 = np.asarray(x_n1, np.float32)
    x2 = np.asarray(x_n2, np.float32)
    ntot = x1.shape[0]
    nloc = ntot // n_cores
    w, betas = prep_weights(params)

    edges0, pad0 = prep_edges(np.asarray(edge_12), n_cores, nloc)
    edges1, pad1 = prep_edges(np.asarray(edge_21), n_cores, nloc)
    pairs, padp, pc = prep_pairs(np.asarray(edge_index), n_cores)
    npairs = np.asarray(edge_index).shape[1]

    cfg = {
        "C": n_cores,
        "NLOC": nloc,
        "FIN": x1.shape[1],
        "L": len(params["layers"]),
        "PAD": [pad0, pad1],
        "PADP": padp,
        "betas": betas,
    }

    in_maps = []
    for c in range(n_cores):
        rows = slice(c * nloc, (c + 1) * nloc)
        m = dict(w)
        m["x0t"] = np.ascontiguousarray(x1[rows].T)
        m["x1t"] = np.ascontiguousarray(x2[rows].T)
        for d, ed in enumerate((edges0, edges1)):
            m[f"src16_{d}"] = ed[c]["src16"]
            m[f"dst16_{d}"] = ed[c]["dst16"]
            m[f"dstloc_{d}"] = ed[c]["dstloc"]
        m["m16"] = pairs[c]["m16"]
        m["d16"] = pairs[c]["d16"]
        in_maps.append(m)

    meta = {"pc": pc, "npairs": npairs, "padp": padp}
    return cfg, in_maps, meta


def unshard(results, meta):
    pc, npairs = meta["pc"], meta["npairs"]
    y = np.empty((npairs, 1), np.float32)
    for c, res in enumerate(results):
        yc = res["y"].T.reshape(-1)  # pair i at [i%128, i//128]
        lo = c * pc
        hi = min(lo + pc, npairs)
        y[lo:hi, 0] = yc[: hi - lo]
    return y


def kernel(x_n1, x_n2, edge_12, edge_21, edge_index, params):
    cfg, in_maps, meta = prepare(x_n1, x_n2, edge_12, edge_21, edge_index, params)
    nc = build_program(cfg)
    res = run_bass_kernel_spmd(nc, in_maps, list(range(cfg["C"])))
    return unshard(res.results, meta)


if __name__ == "__main__":
    import reference

    inputs = reference.setup_inputs()
    y = kernel(**inputs)
    print(y.shape, y.dtype)
